# revision 35
# baseline (speedup 1.0000x reference)
"""Structured butterfly kernel, bf16 datapath (fp32 PSUM accumulation).

x is cast to bf16 AND pre-transposed on host, so the device does only
contiguous DMA loads.  Stages 0-6 (128x128 block-diag) run as
data-stationary bf16 matmuls; stages 7-9 as a PE transpose pass (whose
gather-AP folds the (h,j16) column regrouping) + bf16 matmuls against
16x block-diag(8x8) weights.  All PSUM drains are contiguous full-tile
copies; each subtile's drains run on one engine, alternating ACT/DVE
between subtiles (measured-fastest drain pattern).  Loads ride the
gpsimd SWDGE queue, stores the SP ring.  The output leaves the device
in (g, hp, j16) column order; the host un-permutes with a cheap
reshape/transpose and upcasts bf16 -> f32.
"""

import numpy as np
import ml_dtypes

import concourse.bacc as bacc
import concourse.mybir as mybir
import concourse.tile as tile
from concourse.bass_utils import run_bass_kernel_spmd
from concourse.masks import make_identity

N_CORES = 8
BATCH = 32768
DIM = 1024
STAGES = 10
P = 128
ROWS_PER_CORE = BATCH // N_CORES          # 4096
R_SUPER = 1024                            # rows per load DMA
N_SUPER = ROWS_PER_CORE // R_SUPER        # 4
N_TILES = ROWS_PER_CORE // P              # 32
N_CHUNKS = DIM // P                       # 8
F32 = mybir.dt.float32
BF16 = mybir.dt.bfloat16

_NC = {}


def _stage_product(angles: np.ndarray, stages) -> np.ndarray:
    B = np.eye(DIM, dtype=np.float64)
    k = np.arange(DIM)
    for s in stages:
        stride = 1 << s
        b = k // (2 * stride)
        j = k % stride
        h = (k >> s) & 1
        th = angles[s].astype(np.float64)[b * stride + j]
        C = np.cos(th)
        S = np.where(h == 0, -np.sin(th), np.sin(th))
        B = C[:, None] * B + S[:, None] * B[k ^ stride]
    return B


def _build_weights(angles: np.ndarray):
    """Returns (WA [128, 1024], WB [128, 1024]) bf16.

    WA: per-chunk transposed stage-0..6 product (y keeps natural
    column order h*128 + c).
    WB: for zt partition order (h, j16) of group g (c = g*16 + j16),
    moving columns in (hp, j16) order.
    """
    B_lo = _stage_product(angles, range(7))
    B_hi = _stage_product(angles, range(7, 10))

    WA = np.zeros((P, N_CHUNKS * P), dtype=np.float64)
    for h in range(N_CHUNKS):
        blk = B_lo[h * P:(h + 1) * P, h * P:(h + 1) * P]
        WA[:, h * P:(h + 1) * P] = blk.T

    jj = np.arange(P)
    H = np.zeros((P, 8, 8), dtype=np.float64)
    for hp in range(8):
        for h in range(8):
            H[:, hp, h] = B_hi[hp * P + jj, h * P + jj]

    WB = np.zeros((P, N_CHUNKS * P), dtype=np.float64)
    for g in range(8):
        blk = np.zeros((P, P), dtype=np.float64)
        for j16 in range(16):
            j = 16 * g + j16
            for h in range(8):
                for hp in range(8):
                    blk[h * 16 + j16, hp * 16 + j16] = H[j, hp, h]
        WB[:, g * P:(g + 1) * P] = blk
    return (WA.astype(ml_dtypes.bfloat16), WB.astype(ml_dtypes.bfloat16))


def _build_nc(repeat: int = 1):
    nc = bacc.Bacc(
        "TRN2", target_bir_lowering=False, debug=False, num_devices=N_CORES
    )
    # x arrives pre-transposed: xT[d, r] = x[r, d]
    x_in = nc.dram_tensor(
        "x", [DIM, ROWS_PER_CORE], BF16, kind="ExternalInput"
    ).ap()
    wa_in = nc.dram_tensor("wa", [P, DIM], BF16, kind="ExternalInput").ap()
    wb_in = nc.dram_tensor("wb", [P, DIM], BF16, kind="ExternalInput").ap()
    out = nc.dram_tensor(
        "out", [ROWS_PER_CORE, DIM], BF16, kind="ExternalOutput"
    ).ap()

    # [i, h, r] view of xT: row d = h*128 + i
    xv = x_in.rearrange("(h i) r -> i h r", i=P)

    with tile.TileContext(nc) as tc:
        from contextlib import ExitStack

        with ExitStack() as ctx:
            const = ctx.enter_context(tc.tile_pool(name="const", bufs=1))
            ident = const.tile([P, P], BF16)
            make_identity(nc, ident)

            # 3+2+3 = 8 banks; mmb=3 breaks the MM-B(k+1) <- o-drain(k)
            # recurrence through PSUM bank reuse.
            mma = ctx.enter_context(
                tc.tile_pool(name="mma", bufs=3, space="PSUM")
            )
            tp2 = ctx.enter_context(
                tc.tile_pool(name="tp2", bufs=2, space="PSUM")
            )
            mmb = ctx.enter_context(
                tc.tile_pool(name="mmb", bufs=3, space="PSUM")
            )

            # Consume identity on PE early (single-wait discipline) and
            # trigger the ACT function-table load during startup.
            warm = tp2.tile([P, P], BF16, tag="pt2")
            nc.tensor.transpose(warm[:], ident[:], ident[:])
            warm_act = const.tile([P, 1], BF16)
            nc.scalar.copy(out=warm_act[:], in_=ident[:, 0:1])

            wa_sb = const.tile([P, DIM], BF16)
            nc.sync.dma_start(wa_sb[:], wa_in[:])
            wb_sb = const.tile([P, DIM], BF16)
            nc.sync.dma_start(wb_sb[:], wb_in[:])

            xt_pool = ctx.enter_context(tc.tile_pool(name="xt", bufs=3))
            y_pool = ctx.enter_context(tc.tile_pool(name="y", bufs=4))
            zt_pool = ctx.enter_context(tc.tile_pool(name="zt", bufs=4))
            o_pool = ctx.enter_context(tc.tile_pool(name="o", bufs=4))

            H4 = P * 4

            def cp(use_dve, out_ap, in_ap):
                if use_dve:
                    nc.vector.tensor_copy(out=out_ap, in_=in_ap)
                else:
                    nc.scalar.copy(out=out_ap, in_=in_ap)

            def pass_a(xt, rbase, dve):
                """MM-A for one subtile -> (g, h, j16)-ordered bf16 y.

                The h-regrouping scatter rides the y drain (matmul
                stationary APs must be 2D, so it cannot fold into T2).
                """
                y_t = y_pool.tile([P, DIM], BF16, tag="y")
                y_scatter = y_t[:].rearrange("p (g h j) -> p h g j", g=8, h=8)
                for q in range(2):
                    bank_a = mma.tile([P, H4], F32, tag="pa")
                    for hh in range(4):
                        h = 4 * q + hh
                        nc.tensor.matmul(
                            bank_a[:, hh * P : (hh + 1) * P],
                            xt[:, h, rbase : rbase + P],
                            wa_sb[:, h * P : (h + 1) * P],
                            start=True,
                            stop=True,
                        )
                    cp(
                        dve,
                        y_scatter[:, 4 * q : 4 * q + 4, :, :],
                        bank_a[:].rearrange("p (h g j) -> p h g j", h=4, g=8),
                    )
                return y_t

            def pass_b(y_t, row0, dve):
                """T2 (gather-AP folds the (h,j16) regroup) + MM-B;
                drains contiguous; stores permuted bf16 output."""
                bank_t2 = tp2.tile([P, DIM], BF16, tag="pt2")
                zt_q = zt_pool.tile([P, DIM], BF16, tag="zt")
                for g in range(8):
                    nc.tensor.transpose(
                        bank_t2[:, g * P : (g + 1) * P],
                        y_t[:, g * P : (g + 1) * P],
                        ident[:],
                    )
                # single merged z drain (bf16->bf16, 2x on DVE)
                cp(dve, zt_q[:], bank_t2[:])

                o_t = o_pool.tile([P, DIM], BF16, tag="o")
                for q in range(2):
                    bank_b = mmb.tile([P, H4], F32, tag="pb")
                    for gg in range(4):
                        g = 4 * q + gg
                        nc.tensor.matmul(
                            bank_b[:, gg * P : (gg + 1) * P],
                            zt_q[:, g * P : (g + 1) * P],
                            wb_sb[:, g * P : (g + 1) * P],
                            start=True,
                            stop=True,
                        )
                    cp(dve, o_t[:, q * H4 : (q + 1) * H4], bank_b[:])
                nc.sync.dma_start(out[row0 : row0 + P, :], o_t[:])

            def full_pass():
                # 1-deep software pipeline: pass_b of subtile k-1 runs
                # while pass_a of subtile k fills; each subtile's drains
                # all ride one engine, alternating ACT/DVE by parity.
                k = 0
                pending = []
                for st in range(N_SUPER):
                    r0 = st * R_SUPER
                    # xt[i, h, r] = x[r0 + r, h*128 + i]: one big DMA on
                    # the gpsimd SWDGE queue.
                    xt = xt_pool.tile([P, N_CHUNKS, R_SUPER], BF16, tag="xt")
                    half = R_SUPER // 2
                    nc.gpsimd.dma_start(
                        xt[:, :, 0:half], xv[:, :, r0 : r0 + half]
                    )
                    nc.gpsimd.dma_start(
                        xt[:, :, half:R_SUPER],
                        xv[:, :, r0 + half : r0 + R_SUPER],
                    )
                    for rr in range(R_SUPER // P):
                        dve = k % 2 == 0
                        y_t = pass_a(xt, rr * P, dve)
                        pending.append((y_t, r0 + rr * P, dve))
                        if len(pending) > 1:
                            pass_b(*pending.pop(0))
                        k += 1
                for args in pending:
                    pass_b(*args)

            if repeat > 1:
                # Hardware loop: program size stays constant so large
                # repeat counts (for slope timing) compile fast.
                with tc.For_i(0, repeat):
                    full_pass()
            else:
                full_pass()

    nc.compile()
    return nc


def _get_nc(repeat: int = 1):
    if repeat not in _NC:
        _NC[repeat] = _build_nc(repeat)
    return _NC[repeat]


def prepare_in_maps(x, angles):
    WA, WB = _build_weights(angles)
    xb = x.astype(ml_dtypes.bfloat16)
    shards = xb.reshape(N_CORES, ROWS_PER_CORE, DIM)
    return [
        {
            "x": np.ascontiguousarray(shards[i].T),
            "wa": WA,
            "wb": WB,
        }
        for i in range(N_CORES)
    ]


def unpermute(out_dev: np.ndarray) -> np.ndarray:
    """Device col (g, hp, j16) -> true col (hp, g, j16)."""
    b = out_dev.shape[0]
    return np.ascontiguousarray(
        out_dev.reshape(b, 8, 8, 16).transpose(0, 2, 1, 3).reshape(b, DIM)
    )


def host_ref(x, angles):
    B = _stage_product(angles, range(STAGES))
    return x.astype(np.float64) @ B.T


def kernel(x: np.ndarray, angles: np.ndarray) -> np.ndarray:
    x = np.ascontiguousarray(np.asarray(x, dtype=np.float32))
    angles = np.asarray(angles, dtype=np.float32)
    assert x.shape == (BATCH, DIM), x.shape

    in_maps = prepare_in_maps(x, angles)

    nc = _get_nc()
    res = run_bass_kernel_spmd(nc, in_maps, list(range(N_CORES)))
    out = np.concatenate(
        [unpermute(res.results[i]["out"].astype(np.float32))
         for i in range(N_CORES)],
        axis=0,
    )
    return out


# revision 37
# speedup vs baseline: 1.0717x; 1.0717x over previous
"""Structured butterfly kernel, bf16 datapath (fp32 PSUM accumulation).

x is cast to bf16 AND pre-transposed on host, so the device does only
contiguous DMA loads.  Stages 0-6 (128x128 block-diag) run as
data-stationary bf16 matmuls; stages 7-9 as a PE transpose pass (whose
gather-AP folds the (h,j16) column regrouping) + bf16 matmuls against
16x block-diag(8x8) weights.  All PSUM drains are contiguous full-tile
copies; each subtile's drains run on one engine, alternating ACT/DVE
between subtiles (measured-fastest drain pattern).  Loads ride the
gpsimd SWDGE queue, stores the SP ring.  The output leaves the device
in (g, hp, j16) column order; the host un-permutes with a cheap
reshape/transpose and upcasts bf16 -> f32.
"""

import numpy as np
import ml_dtypes

import concourse.bacc as bacc
import concourse.mybir as mybir
import concourse.tile as tile
from concourse.bass_utils import run_bass_kernel_spmd
from concourse.masks import make_identity

N_CORES = 8
BATCH = 32768
DIM = 1024
STAGES = 10
P = 128
ROWS_PER_CORE = BATCH // N_CORES          # 4096
R_SUPER = 1024                            # rows per load DMA
N_SUPER = ROWS_PER_CORE // R_SUPER        # 4
N_TILES = ROWS_PER_CORE // P              # 32
N_CHUNKS = DIM // P                       # 8
F32 = mybir.dt.float32
BF16 = mybir.dt.bfloat16

_NC = {}


def _stage_product(angles: np.ndarray, stages) -> np.ndarray:
    B = np.eye(DIM, dtype=np.float64)
    k = np.arange(DIM)
    for s in stages:
        stride = 1 << s
        b = k // (2 * stride)
        j = k % stride
        h = (k >> s) & 1
        th = angles[s].astype(np.float64)[b * stride + j]
        C = np.cos(th)
        S = np.where(h == 0, -np.sin(th), np.sin(th))
        B = C[:, None] * B + S[:, None] * B[k ^ stride]
    return B


def _build_weights(angles: np.ndarray):
    """Returns (WA [128, 1024], WB [128, 1024]) bf16.

    WA: per-chunk transposed stage-0..6 product (y keeps natural
    column order h*128 + c).
    WB: for zt partition order (h, j16) of group g (c = g*16 + j16),
    moving columns in (hp, j16) order.
    """
    B_lo = _stage_product(angles, range(7))
    B_hi = _stage_product(angles, range(7, 10))

    WA = np.zeros((P, N_CHUNKS * P), dtype=np.float64)
    for h in range(N_CHUNKS):
        blk = B_lo[h * P:(h + 1) * P, h * P:(h + 1) * P]
        WA[:, h * P:(h + 1) * P] = blk.T

    jj = np.arange(P)
    H = np.zeros((P, 8, 8), dtype=np.float64)
    for hp in range(8):
        for h in range(8):
            H[:, hp, h] = B_hi[hp * P + jj, h * P + jj]

    WB = np.zeros((P, N_CHUNKS * P), dtype=np.float64)
    for g in range(8):
        blk = np.zeros((P, P), dtype=np.float64)
        for j16 in range(16):
            j = 16 * g + j16
            for h in range(8):
                for hp in range(8):
                    blk[h * 16 + j16, hp * 16 + j16] = H[j, hp, h]
        WB[:, g * P:(g + 1) * P] = blk
    return (WA.astype(ml_dtypes.bfloat16), WB.astype(ml_dtypes.bfloat16))


def _build_nc(repeat: int = 1):
    nc = bacc.Bacc(
        "TRN2", target_bir_lowering=False, debug=False, num_devices=N_CORES
    )
    # x arrives pre-transposed: xT[d, r] = x[r, d]
    x_in = nc.dram_tensor(
        "x", [DIM, ROWS_PER_CORE], BF16, kind="ExternalInput"
    ).ap()
    wa_in = nc.dram_tensor("wa", [P, DIM], BF16, kind="ExternalInput").ap()
    wb_in = nc.dram_tensor("wb", [P, DIM], BF16, kind="ExternalInput").ap()
    out = nc.dram_tensor(
        "out", [ROWS_PER_CORE, DIM], BF16, kind="ExternalOutput"
    ).ap()

    # [i, h, r] view of xT: row d = h*128 + i
    xv = x_in.rearrange("(h i) r -> i h r", i=P)

    with tile.TileContext(nc) as tc:
        from contextlib import ExitStack

        with ExitStack() as ctx:
            const = ctx.enter_context(tc.tile_pool(name="const", bufs=1))
            ident = const.tile([P, P], BF16)
            make_identity(nc, ident)

            # 3+2+3 = 8 banks; mmb=3 breaks the MM-B(k+1) <- o-drain(k)
            # recurrence through PSUM bank reuse.
            mma = ctx.enter_context(
                tc.tile_pool(name="mma", bufs=3, space="PSUM")
            )
            tp2 = ctx.enter_context(
                tc.tile_pool(name="tp2", bufs=2, space="PSUM")
            )
            mmb = ctx.enter_context(
                tc.tile_pool(name="mmb", bufs=3, space="PSUM")
            )

            # Consume identity on PE early (single-wait discipline) and
            # trigger the ACT function-table load during startup.
            warm = tp2.tile([P, P], BF16, tag="pt2")
            nc.tensor.transpose(warm[:], ident[:], ident[:])
            warm_act = const.tile([P, 1], BF16)
            nc.scalar.copy(out=warm_act[:], in_=ident[:, 0:1])

            wa_sb = const.tile([P, DIM], BF16)
            nc.sync.dma_start(wa_sb[:], wa_in[:])
            wb_sb = const.tile([P, DIM], BF16)
            nc.sync.dma_start(wb_sb[:], wb_in[:])

            xt_pool = ctx.enter_context(tc.tile_pool(name="xt", bufs=3))
            y_pool = ctx.enter_context(tc.tile_pool(name="y", bufs=4))
            zt_pool = ctx.enter_context(tc.tile_pool(name="zt", bufs=4))
            o_pool = ctx.enter_context(tc.tile_pool(name="o", bufs=4))

            H4 = P * 4

            def cp(use_dve, out_ap, in_ap):
                if use_dve:
                    nc.vector.tensor_copy(out=out_ap, in_=in_ap)
                else:
                    nc.scalar.copy(out=out_ap, in_=in_ap)

            def pass_a(xt, rbase, dve):
                """MM-A for one subtile -> (g, h, j16)-ordered bf16 y.

                The h-regrouping scatter rides the y drain (matmul
                stationary APs must be 2D, so it cannot fold into T2).
                """
                y_t = y_pool.tile([P, DIM], BF16, tag="y")
                y_scatter = y_t[:].rearrange("p (g h j) -> p h g j", g=8, h=8)
                for q in range(2):
                    bank_a = mma.tile([P, H4], F32, tag="pa")
                    for hh in range(4):
                        h = 4 * q + hh
                        nc.tensor.matmul(
                            bank_a[:, hh * P : (hh + 1) * P],
                            xt[:, h, rbase : rbase + P],
                            wa_sb[:, h * P : (h + 1) * P],
                            start=True,
                            stop=True,
                        )
                    cp(
                        dve,
                        y_scatter[:, 4 * q : 4 * q + 4, :, :],
                        bank_a[:].rearrange("p (h g j) -> p h g j", h=4, g=8),
                    )
                return y_t

            def pass_b(y_t, row0, dve):
                """T2 (gather-AP folds the (h,j16) regroup) + MM-B;
                drains contiguous; stores permuted bf16 output."""
                bank_t2 = tp2.tile([P, DIM], BF16, tag="pt2")
                zt_q = zt_pool.tile([P, DIM], BF16, tag="zt")
                for g in range(8):
                    nc.tensor.transpose(
                        bank_t2[:, g * P : (g + 1) * P],
                        y_t[:, g * P : (g + 1) * P],
                        ident[:],
                    )
                # z always drains on DVE: bf16->bf16 runs at 2x there,
                # but at 1x on ACT's InstActivation.
                cp(True, zt_q[:], bank_t2[:])

                o_t = o_pool.tile([P, DIM], BF16, tag="o")
                for q in range(2):
                    bank_b = mmb.tile([P, H4], F32, tag="pb")
                    for gg in range(4):
                        g = 4 * q + gg
                        nc.tensor.matmul(
                            bank_b[:, gg * P : (gg + 1) * P],
                            zt_q[:, g * P : (g + 1) * P],
                            wb_sb[:, g * P : (g + 1) * P],
                            start=True,
                            stop=True,
                        )
                    # DVE-subtiles hand their q1 o-half to ACT, paying
                    # back the z drains DVE absorbs from ACT-subtiles.
                    o_dve = dve if q == 0 else False
                    cp(o_dve, o_t[:, q * H4 : (q + 1) * H4], bank_b[:])
                nc.sync.dma_start(out[row0 : row0 + P, :], o_t[:])

            def full_pass():
                # 1-deep software pipeline: pass_b of subtile k-1 runs
                # while pass_a of subtile k fills; each subtile's drains
                # all ride one engine, alternating ACT/DVE by parity.
                k = 0
                pending = []
                for st in range(N_SUPER):
                    r0 = st * R_SUPER
                    # xt[i, h, r] = x[r0 + r, h*128 + i]: one big DMA on
                    # the gpsimd SWDGE queue.
                    xt = xt_pool.tile([P, N_CHUNKS, R_SUPER], BF16, tag="xt")
                    half = R_SUPER // 2
                    nc.gpsimd.dma_start(
                        xt[:, :, 0:half], xv[:, :, r0 : r0 + half]
                    )
                    nc.gpsimd.dma_start(
                        xt[:, :, half:R_SUPER],
                        xv[:, :, r0 + half : r0 + R_SUPER],
                    )
                    for rr in range(R_SUPER // P):
                        dve = k % 2 == 0
                        y_t = pass_a(xt, rr * P, dve)
                        pending.append((y_t, r0 + rr * P, dve))
                        if len(pending) > 1:
                            pass_b(*pending.pop(0))
                        k += 1
                for args in pending:
                    pass_b(*args)

            if repeat > 1:
                # Hardware loop: program size stays constant so large
                # repeat counts (for slope timing) compile fast.
                with tc.For_i(0, repeat):
                    full_pass()
            else:
                full_pass()

    nc.compile()
    return nc


def _get_nc(repeat: int = 1):
    if repeat not in _NC:
        _NC[repeat] = _build_nc(repeat)
    return _NC[repeat]


def prepare_in_maps(x, angles):
    WA, WB = _build_weights(angles)
    xb = x.astype(ml_dtypes.bfloat16)
    shards = xb.reshape(N_CORES, ROWS_PER_CORE, DIM)
    return [
        {
            "x": np.ascontiguousarray(shards[i].T),
            "wa": WA,
            "wb": WB,
        }
        for i in range(N_CORES)
    ]


def unpermute(out_dev: np.ndarray) -> np.ndarray:
    """Device col (g, hp, j16) -> true col (hp, g, j16)."""
    b = out_dev.shape[0]
    return np.ascontiguousarray(
        out_dev.reshape(b, 8, 8, 16).transpose(0, 2, 1, 3).reshape(b, DIM)
    )


def host_ref(x, angles):
    B = _stage_product(angles, range(STAGES))
    return x.astype(np.float64) @ B.T


def kernel(x: np.ndarray, angles: np.ndarray) -> np.ndarray:
    x = np.ascontiguousarray(np.asarray(x, dtype=np.float32))
    angles = np.asarray(angles, dtype=np.float32)
    assert x.shape == (BATCH, DIM), x.shape

    in_maps = prepare_in_maps(x, angles)

    nc = _get_nc()
    res = run_bass_kernel_spmd(nc, in_maps, list(range(N_CORES)))
    out = np.concatenate(
        [unpermute(res.results[i]["out"].astype(np.float32))
         for i in range(N_CORES)],
        axis=0,
    )
    return out


# revision 38
# speedup vs baseline: 1.1950x; 1.1150x over previous
"""Structured butterfly kernel, bf16 datapath (fp32 PSUM accumulation).

x is cast to bf16 AND pre-transposed on host, so the device does only
contiguous DMA loads.  Stages 0-6 (128x128 block-diag) run as
data-stationary bf16 matmuls; stages 7-9 as a PE transpose pass (whose
gather-AP folds the (h,j16) column regrouping) + bf16 matmuls against
16x block-diag(8x8) weights.  All PSUM drains are contiguous full-tile
copies; each subtile's drains run on one engine, alternating ACT/DVE
between subtiles (measured-fastest drain pattern).  Loads ride the
gpsimd SWDGE queue, stores the SP ring.  The output leaves the device
in (g, hp, j16) column order; the host un-permutes with a cheap
reshape/transpose and upcasts bf16 -> f32.
"""

import numpy as np
import ml_dtypes

import concourse.bacc as bacc
import concourse.mybir as mybir
import concourse.tile as tile
from concourse.bass_utils import run_bass_kernel_spmd
from concourse.masks import make_identity

N_CORES = 8
BATCH = 32768
DIM = 1024
STAGES = 10
P = 128
ROWS_PER_CORE = BATCH // N_CORES          # 4096
R_SUPER = 1024                            # rows per load DMA
N_SUPER = ROWS_PER_CORE // R_SUPER        # 4
N_TILES = ROWS_PER_CORE // P              # 32
N_CHUNKS = DIM // P                       # 8
F32 = mybir.dt.float32
BF16 = mybir.dt.bfloat16

_NC = {}


def _stage_product(angles: np.ndarray, stages) -> np.ndarray:
    B = np.eye(DIM, dtype=np.float64)
    k = np.arange(DIM)
    for s in stages:
        stride = 1 << s
        b = k // (2 * stride)
        j = k % stride
        h = (k >> s) & 1
        th = angles[s].astype(np.float64)[b * stride + j]
        C = np.cos(th)
        S = np.where(h == 0, -np.sin(th), np.sin(th))
        B = C[:, None] * B + S[:, None] * B[k ^ stride]
    return B


def _build_weights(angles: np.ndarray):
    """Returns (WA [128, 1024], WB [128, 1024]) bf16.

    WA: per-chunk transposed stage-0..6 product (y keeps natural
    column order h*128 + c).
    WB: for zt partition order (h, j16) of group g (c = g*16 + j16),
    moving columns in (hp, j16) order.
    """
    B_lo = _stage_product(angles, range(7))
    B_hi = _stage_product(angles, range(7, 10))

    WA = np.zeros((P, N_CHUNKS * P), dtype=np.float64)
    for h in range(N_CHUNKS):
        blk = B_lo[h * P:(h + 1) * P, h * P:(h + 1) * P]
        WA[:, h * P:(h + 1) * P] = blk.T

    jj = np.arange(P)
    H = np.zeros((P, 8, 8), dtype=np.float64)
    for hp in range(8):
        for h in range(8):
            H[:, hp, h] = B_hi[hp * P + jj, h * P + jj]

    WB = np.zeros((P, N_CHUNKS * P), dtype=np.float64)
    for g in range(8):
        blk = np.zeros((P, P), dtype=np.float64)
        for j16 in range(16):
            j = 16 * g + j16
            for h in range(8):
                for hp in range(8):
                    blk[h * 16 + j16, hp * 16 + j16] = H[j, hp, h]
        WB[:, g * P:(g + 1) * P] = blk
    return (WA.astype(ml_dtypes.bfloat16), WB.astype(ml_dtypes.bfloat16))


def _build_nc(repeat: int = 1):
    nc = bacc.Bacc(
        "TRN2", target_bir_lowering=False, debug=False, num_devices=N_CORES
    )
    # x arrives pre-transposed: xT[d, r] = x[r, d]
    x_in = nc.dram_tensor(
        "x", [DIM, ROWS_PER_CORE], BF16, kind="ExternalInput"
    ).ap()
    wa_in = nc.dram_tensor("wa", [P, DIM], BF16, kind="ExternalInput").ap()
    wb_in = nc.dram_tensor("wb", [P, DIM], BF16, kind="ExternalInput").ap()
    out = nc.dram_tensor(
        "out", [ROWS_PER_CORE, DIM], BF16, kind="ExternalOutput"
    ).ap()

    # [i, h, r] view of xT: row d = h*128 + i
    xv = x_in.rearrange("(h i) r -> i h r", i=P)

    with tile.TileContext(nc) as tc:
        from contextlib import ExitStack

        with ExitStack() as ctx:
            const = ctx.enter_context(tc.tile_pool(name="const", bufs=1))
            ident = const.tile([P, P], BF16)
            make_identity(nc, ident)

            # 3+2+3 = 8 banks; mmb=3 breaks the MM-B(k+1) <- o-drain(k)
            # recurrence through PSUM bank reuse.
            mma = ctx.enter_context(
                tc.tile_pool(name="mma", bufs=2, space="PSUM")
            )
            tp2 = ctx.enter_context(
                tc.tile_pool(name="tp2", bufs=2, space="PSUM")
            )
            mmb = ctx.enter_context(
                tc.tile_pool(name="mmb", bufs=2, space="PSUM")
            )

            # Consume identity on PE early (single-wait discipline) and
            # trigger the ACT function-table load during startup.
            warm = tp2.tile([P, P], BF16, tag="pt2")
            nc.tensor.transpose(warm[:], ident[:], ident[:])
            warm_act = const.tile([P, 1], BF16)
            nc.scalar.copy(out=warm_act[:], in_=ident[:, 0:1])

            wa_sb = const.tile([P, DIM], BF16)
            nc.sync.dma_start(wa_sb[:], wa_in[:])
            wb_sb = const.tile([P, DIM], BF16)
            nc.sync.dma_start(wb_sb[:], wb_in[:])

            xt_pool = ctx.enter_context(tc.tile_pool(name="xt", bufs=3))
            y_pool = ctx.enter_context(tc.tile_pool(name="y", bufs=4))
            zt_pool = ctx.enter_context(tc.tile_pool(name="zt", bufs=4))
            o_pool = ctx.enter_context(tc.tile_pool(name="o", bufs=4))

            H4 = P * 4

            def cp(use_dve, out_ap, in_ap):
                if use_dve:
                    nc.vector.tensor_copy(out=out_ap, in_=in_ap)
                else:
                    nc.scalar.copy(out=out_ap, in_=in_ap)

            def pass_a(xt, rbase, dve):
                """MM-A for one subtile -> (g, h, j16)-ordered bf16 y.

                The h-regrouping scatter rides the y drain (matmul
                stationary APs must be 2D, so it cannot fold into T2).
                """
                y_t = y_pool.tile([P, DIM], BF16, tag="y")
                y_scatter = y_t[:].rearrange("p (g h j) -> p h g j", g=8, h=8)
                # 2-bank mma tile: all 8 matmuls land in one tile so the
                # y drain is a single merged op.
                bank_a = mma.tile([P, DIM], F32, tag="pa")
                for h in range(8):
                    nc.tensor.matmul(
                        bank_a[:, h * P : (h + 1) * P],
                        xt[:, h, rbase : rbase + P],
                        wa_sb[:, h * P : (h + 1) * P],
                        start=True,
                        stop=True,
                    )
                cp(
                    dve,
                    y_scatter[:],
                    bank_a[:].rearrange("p (h g j) -> p h g j", h=8, g=8),
                )
                return y_t

            def pass_b(y_t, row0, dve):
                """T2 (gather-AP folds the (h,j16) regroup) + MM-B;
                drains contiguous; stores permuted bf16 output."""
                bank_t2 = tp2.tile([P, DIM], BF16, tag="pt2")
                zt_q = zt_pool.tile([P, DIM], BF16, tag="zt")
                for g in range(8):
                    nc.tensor.transpose(
                        bank_t2[:, g * P : (g + 1) * P],
                        y_t[:, g * P : (g + 1) * P],
                        ident[:],
                    )
                # z always drains on DVE: bf16->bf16 runs at 2x there,
                # but at 1x on ACT's InstActivation.
                cp(True, zt_q[:], bank_t2[:])

                o_t = o_pool.tile([P, DIM], BF16, tag="o")
                for q in range(2):
                    bank_b = mmb.tile([P, H4], F32, tag="pb")
                    for gg in range(4):
                        g = 4 * q + gg
                        nc.tensor.matmul(
                            bank_b[:, gg * P : (gg + 1) * P],
                            zt_q[:, g * P : (g + 1) * P],
                            wb_sb[:, g * P : (g + 1) * P],
                            start=True,
                            stop=True,
                        )
                    # DVE-subtiles hand their q1 o-half to ACT, paying
                    # back the z drains DVE absorbs from ACT-subtiles.
                    o_dve = dve if q == 0 else False
                    cp(o_dve, o_t[:, q * H4 : (q + 1) * H4], bank_b[:])
                nc.sync.dma_start(out[row0 : row0 + P, :], o_t[:])

            def full_pass():
                # 1-deep software pipeline: pass_b of subtile k-1 runs
                # while pass_a of subtile k fills; each subtile's drains
                # all ride one engine, alternating ACT/DVE by parity.
                k = 0
                pending = []
                for st in range(N_SUPER):
                    r0 = st * R_SUPER
                    # xt[i, h, r] = x[r0 + r, h*128 + i]: one big DMA on
                    # the gpsimd SWDGE queue.
                    xt = xt_pool.tile([P, N_CHUNKS, R_SUPER], BF16, tag="xt")
                    half = R_SUPER // 2
                    nc.gpsimd.dma_start(
                        xt[:, :, 0:half], xv[:, :, r0 : r0 + half]
                    )
                    nc.gpsimd.dma_start(
                        xt[:, :, half:R_SUPER],
                        xv[:, :, r0 + half : r0 + R_SUPER],
                    )
                    for rr in range(R_SUPER // P):
                        dve = k % 2 == 0
                        y_t = pass_a(xt, rr * P, dve)
                        pending.append((y_t, r0 + rr * P, dve))
                        if len(pending) > 1:
                            pass_b(*pending.pop(0))
                        k += 1
                for args in pending:
                    pass_b(*args)

            if repeat > 1:
                # Hardware loop: program size stays constant so large
                # repeat counts (for slope timing) compile fast.
                with tc.For_i(0, repeat):
                    full_pass()
            else:
                full_pass()

    nc.compile()
    return nc


def _get_nc(repeat: int = 1):
    if repeat not in _NC:
        _NC[repeat] = _build_nc(repeat)
    return _NC[repeat]


def prepare_in_maps(x, angles):
    WA, WB = _build_weights(angles)
    xb = x.astype(ml_dtypes.bfloat16)
    shards = xb.reshape(N_CORES, ROWS_PER_CORE, DIM)
    return [
        {
            "x": np.ascontiguousarray(shards[i].T),
            "wa": WA,
            "wb": WB,
        }
        for i in range(N_CORES)
    ]


def unpermute(out_dev: np.ndarray) -> np.ndarray:
    """Device col (g, hp, j16) -> true col (hp, g, j16)."""
    b = out_dev.shape[0]
    return np.ascontiguousarray(
        out_dev.reshape(b, 8, 8, 16).transpose(0, 2, 1, 3).reshape(b, DIM)
    )


def host_ref(x, angles):
    B = _stage_product(angles, range(STAGES))
    return x.astype(np.float64) @ B.T


def kernel(x: np.ndarray, angles: np.ndarray) -> np.ndarray:
    x = np.ascontiguousarray(np.asarray(x, dtype=np.float32))
    angles = np.asarray(angles, dtype=np.float32)
    assert x.shape == (BATCH, DIM), x.shape

    in_maps = prepare_in_maps(x, angles)

    nc = _get_nc()
    res = run_bass_kernel_spmd(nc, in_maps, list(range(N_CORES)))
    out = np.concatenate(
        [unpermute(res.results[i]["out"].astype(np.float32))
         for i in range(N_CORES)],
        axis=0,
    )
    return out
